# revision 49
# baseline (speedup 1.0000x reference)
"""Trainium2 Bass kernel for MixedPerformerAttention (B=2,S=2048,D=2048,H=16).

Sharding: 8 cores = 2 batches x 4 head-slots. Core c (b=c//4, j=c%4) owns
performer heads {2j, 2j+1} (kv head j) and softmax heads {8+2j, 8+2j+1}
(kv head 4+j), plus the matching Wq/Wk/Wv rows and Wo columns. Each core
computes a [S, D] partial output projection; the host sums 4 partials/batch.

All matmul operands are bf16 (fp32 PSUM accumulation). The two performer
heads share their GQA kv head, so k-features, pk and the entire kv prefix
state are computed once and reused by both heads; the q-side processes both
heads per chunk through merged [128,256] tiles (h-major) to amortize DVE
per-instruction overhead. Reciprocals run on DVE (vector.reciprocal), and
the performer per-token 1/den broadcast is a rank-1 f32r matmul instead of
gpsimd partition_broadcast.

Emission is software-pipelined: scores exp(i) overlaps AV(i-1) matmuls; the
performer chunks run FEAT/BIAS/ATTN stages with the O-projection of window
J-1 interleaved per chunk (sb=t) so its dense matmuls hide the performer's
DVE/ACT chains; for the last window its own O-projection chases each chunk.

The performer branch reproduces the reference's exact stabilizers (per-token
q-stab + per-(b,h) global k-stab) so the EPS=1e-6 denominator guard matches;
stabk is computed on the host at runtime and shipped in `nbinit`.

PSUM (8 banks): pp ring x2 (projections / scores / performer features),
av x2 (softmax accumulators), sm ring x2 (transposes/aT/kvc/num/bb/pso),
dacc0 (softmax den h0 row + performer den row), dacc1 (softmax den h1).
"""

import sys

sys.path.insert(0, "/opt/trn_rl_repo")

import numpy as np

import concourse.bass as bass
import concourse.tile as tile
from concourse import bacc, mybir
from concourse._compat import with_exitstack

F32 = mybir.dt.float32
F32R = mybir.dt.float32r
BF16 = mybir.dt.bfloat16
AF = mybir.ActivationFunctionType
AX = mybir.AxisListType
ALU = mybir.AluOpType

B, S, D = 2, 2048, 2048
H, KVH, HD = 16, 8, 128
NPH, M, C = 8, 128, 128
SCALE = HD ** -0.5
EPS = 1e-6
LNM = float(np.log(np.sqrt(M)))
HDQ = HD ** -0.25

NJ, JW, ND, NB = 4, 512, 16, 16


@with_exitstack
def _emit(ctx, tc, aps, debug=False):
    nc = tc.nc
    hsT, wq, wk, wv, wo = aps["hsT"], aps["wq"], aps["wk"], aps["wv"], aps["wo"]
    out = aps["out"]

    pers = ctx.enter_context(tc.tile_pool(name="pers", bufs=1))
    hst_p = ctx.enter_context(tc.tile_pool(name="hst", bufs=2))
    rot_p = ctx.enter_context(tc.tile_pool(name="rot", bufs=2))
    qt_p = ctx.enter_context(tc.tile_pool(name="qt", bufs=2))
    pt_p = ctx.enter_context(tc.tile_pool(name="pt", bufs=2))
    at_p = ctx.enter_context(tc.tile_pool(name="at", bufs=2))
    sm_p = ctx.enter_context(tc.tile_pool(name="sm", bufs=2))
    ost_p = ctx.enter_context(tc.tile_pool(name="ost", bufs=2))
    psp = ctx.enter_context(tc.tile_pool(name="psp", bufs=1, space="PSUM"))

    def ppt(shape=None, tag="pp"):
        return psp.tile(shape or [128, JW], F32, name=tag, tag="pp", bufs=2)

    def smt(shape, dt, name):
        return psp.tile(shape, dt, name=name, tag="sm", bufs=2)

    mm = nc.tensor.matmul

    # ---- weights + J0 activations, batched DMAs interleaved so the first
    # projection matmuls start after one chunk; tiny constants (needed only
    # by B/C) issue after the first compute-critical chunks ----
    wq_t = pers.tile([128, ND * 512], BF16, name="wq_t", tag="wq")
    wk_t = pers.tile([128, ND * 256], BF16, name="wk_t", tag="wk")
    wv_t = pers.tile([128, ND * 256], BF16, name="wv_t", tag="wv")
    wo_t = pers.tile([128, 4 * D], BF16, name="wo_t", tag="wo")
    hst0 = hst_p.tile([128, ND * JW], BF16, name="hst", tag="hst")
    wq_src = wq.rearrange("(d p) c -> p d c", p=128)
    hs_3 = hst0.rearrange("p (d s) -> p d s", d=ND)
    for dd in range(4):
        sl = slice(dd * 4, (dd + 1) * 4)
        nc.sync.dma_start(wq_t.rearrange("p (d c) -> p d c", d=ND)[:, sl, :],
                          wq_src[:, sl, :])
        nc.sync.dma_start(hs_3[:, sl, :],
                          hsT[:, 0:JW].rearrange("(d p) s -> p d s", p=128)[:, sl, :])
    co0 = rot_p.tile([128, JW], BF16, name="cos", tag="cos")
    si0 = rot_p.tile([128, JW], BF16, name="sin", tag="sin")
    nc.sync.dma_start(co0[:], aps["cost"][:, 0:JW])
    nc.sync.dma_start(si0[:], aps["sintn"][:, 0:JW])
    nc.sync.dma_start(wk_t.rearrange("p (d c) -> p d c", d=ND),
                      wk.rearrange("(d p) c -> p d c", p=128))
    nc.sync.dma_start(wv_t.rearrange("p (d c) -> p d c", d=ND),
                      wv.rearrange("(d p) c -> p d c", p=128))
    omgx = pers.tile([128, 128], BF16, name="omgx", tag="omgx")
    nc.sync.dma_start(omgx[:], aps["omgx"][:])
    cons2 = pers.tile([128, 2], BF16, name="cons2", tag="cons2")
    nc.sync.dma_start(cons2[:], aps["cons2"][:])
    ident = pers.tile([128, 128], BF16, name="ident", tag="ident")
    nc.sync.dma_start(ident[:], aps["ident"][:])
    trimask = pers.tile([128, 128], BF16, name="trimask", tag="trimask")
    nc.sync.dma_start(trimask[:], aps["trimask"][:])
    onescol = pers.tile([128, 1], BF16, name="onescol", tag="onescol")
    nc.sync.dma_start(onescol[:], aps["onescol"][:])
    onesr = pers.tile([1, 128], BF16, name="onesr", tag="onesr")
    nc.sync.dma_start(onesr[:], aps["onesr"][:])
    nbinit = pers.tile([128, 4], F32, name="nbinit", tag="nbinit")
    nc.sync.dma_start(nbinit[:], aps["nbinit"][:])
    nc.sync.dma_start(wo_t.rearrange("p (i c) -> p i c", i=4),
                      wo.rearrange("(i p) c -> p i c", p=128))

    # ---- persistent K/V and performer state ----
    ktp = pers.tile([128, 2048], BF16, name="ktp", tag="ktp")
    kts = pers.tile([128, 2048], BF16, name="kts", tag="kts")
    vp = [pers.tile([128, 132], BF16, name=f"vp{i}", tag=f"vp{i}") for i in range(NB)]
    vs = [pers.tile([128, 128], BF16, name=f"vs{i}", tag=f"vs{i}") for i in range(NB)]
    for i in range(NB):
        nc.vector.memset(vp[i][:, 128:129], 1.0)
    kv_bf = sm_p.tile([128, 132], BF16, name="kvbf", tag="kvbf", bufs=2)
    nc.vector.memset(kv_bf[:, 0:129], 0.0)

    # softmax denominator banks, one per head
    dh = [psp.tile([128, 512], F32, name=f"dacc{h}", tag=f"dacc{h}", bufs=1)
          for h in range(2)]
    dn_sl = [dh[h][0:1, :] for h in range(2)]

    def rotary(ps, dst, co, si):
        # dst = ps*cos + rot_half(ps)*sin; sintn has [-s; s] baked in. The
        # half-swapped products read PSUM directly (mixed PSUM/SB operands
        # are exempt from the same-base-partition rule).
        tmp = rot_p.tile([128, JW], BF16, name="rtmp", tag="rtmp", bufs=2)
        nc.vector.tensor_mul(tmp[0:64, :], ps[64:128, :], si[0:64, :])
        nc.vector.tensor_mul(tmp[64:128, :], ps[0:64, :], si[64:128, :])
        pc = rot_p.tile([128, JW], BF16, name="pc", tag="pc", bufs=2)
        nc.scalar.copy(pc[:], ps[:])
        nc.vector.tensor_mul(dst, pc[:], co[:])
        nc.vector.tensor_add(dst, dst, tmp[:])

    def oproj_sb(Jp, at3, sb):
        atp01p, ats0p, ats1p = at3
        s0p = Jp * JW
        o = ost_p.tile([128, D], BF16, name="ost", tag="ost", bufs=2)
        asl = [atp01p[:, sb * 128:sb * 128 + 128],
               atp01p[:, 512 + sb * 128:512 + sb * 128 + 128],
               ats0p[:, sb * 128:(sb + 1) * 128],
               ats1p[:, sb * 128:(sb + 1) * 128]]
        for oc in range(4):
            pso = smt([128, JW], F32, "pso")
            for i in range(4):
                mm(pso[:], asl[i], wo_t[:, i * D + oc * 512:i * D + (oc + 1) * 512],
                   start=(i == 0), stop=(i == 3))
            if oc % 2 == 0:
                nc.vector.tensor_copy(o[:, oc * 512:(oc + 1) * 512], pso[:])
            else:
                nc.scalar.copy(o[:, oc * 512:(oc + 1) * 512], pso[:])
                half = slice((oc - 1) * 512, (oc + 1) * 512)
                nc.sync.dma_start(
                    out[s0p + sb * 128:s0p + (sb + 1) * 128, half], o[:, half])

    prev_at = None

    pend = None  # deferred last performer unit of the previous window

    for J in range(NJ):
        s0 = J * JW
        if J == 0:
            hst, co, si = hst0, co0, si0
        else:
            hst, co, si = hst_n, co_n, si_n

        # ================= A: projections =================
        # The previous window's last performer unit is emitted between
        # projection groups: its cross-engine chains resolved long ago, so
        # it can never head-of-line-block this window's dense matmuls.
        qt01 = qt_p.tile([128, 1024], BF16, name="qt01", tag="qt01", bufs=2)
        qt2 = qt_p.tile([128, JW], BF16, name="qt2", tag="qt2", bufs=2)
        qt3 = qt_p.tile([128, JW], BF16, name="qt3", tag="qt3", bufs=2)
        qdst = [qt01[:, 0:512], qt01[:, 512:1024], qt2[:], qt3[:]]
        for g in range(4):
            ps = ppt()
            for d in range(ND):
                mm(ps[:], wq_t[:, d * 512 + g * 128:d * 512 + (g + 1) * 128],
                   hst[:, d * JW:(d + 1) * JW], start=(d == 0), stop=(d == ND - 1))
            rotary(ps, qdst[g], co, si)
            if g == 1 and pend is not None:
                pend[0]()  # attn of previous window's t=3
        for g in range(2):
            ps = ppt()
            for d in range(ND):
                mm(ps[:], wk_t[:, d * 256 + g * 128:d * 256 + (g + 1) * 128],
                   hst[:, d * JW:(d + 1) * JW], start=(d == 0), stop=(d == ND - 1))
            kt = ktp if g == 0 else kts
            rotary(ps, kt[:, s0:s0 + JW], co, si)
            if g == 1 and pend is not None:
                pend[1]()  # fin of previous window's t=3
                pend = None
        def vproj_sb(Jv, hstv, sb, filler=False):
            blk = 4 * Jv + sb
            # filler blocks run inside the performer section and must not
            # touch the pp ring (its tiles have not been consumed yet there)
            ps = smt([128, 256], F32, "psv") if filler else ppt([128, 256])
            for d in range(ND):
                mm(ps[:], hstv[:, d * JW + sb * 128:d * JW + (sb + 1) * 128],
                   wv_t[:, d * 256:(d + 1) * 256], start=(d == 0), stop=(d == ND - 1))
            nc.scalar.copy(vp[blk][:, 0:128], ps[:, 0:128])
            nc.scalar.copy(vs[blk][:], ps[:, 128:256])

        if J != 1:  # window 1's V-projections run as window 0's C filler
            for sb in range(4):
                vproj_sb(J, hst, sb)

        # prefetch next window's activations while B/C run
        if J + 1 < NJ:
            s1 = (J + 1) * JW
            hst_n = hst_p.tile([128, ND * JW], BF16, name="hst", tag="hst")
            nc.sync.dma_start(
                hst_n.rearrange("p (d s) -> p d s", d=ND),
                hsT[:, s1:s1 + JW].rearrange("(d p) s -> p d s", p=128))
            co_n = rot_p.tile([128, JW], BF16, name="cos", tag="cos")
            si_n = rot_p.tile([128, JW], BF16, name="sin", tag="sin")
            nc.sync.dma_start(co_n[:], aps["cost"][:, s1:s1 + JW])
            nc.sync.dma_start(si_n[:], aps["sintn"][:, s1:s1 + JW])

        # ================= B: softmax heads =================
        nblk = 4 * J + 4
        av = [psp.tile([128, JW], F32, name=f"av{h}", tag="av", bufs=2)
              for h in range(2)]
        pts = {}

        def st_exp(i):
            t = i - 4 * J  # >= 0 on diagonal blocks
            q0 = max(t, 0) * 128
            for h in range(2):
                st = ppt()
                mm(st[:, q0:JW], kts[:, i * 128:(i + 1) * 128],
                   (qt2 if h == 0 else qt3)[:, q0:JW], start=True, stop=True)
                pth = pt_p.tile([128, JW], BF16, name=f"pt{h}", tag=f"pt{h}",
                                bufs=2)
                nc.scalar.activation(pth[:, q0:JW], st[:, q0:JW], AF.Exp,
                                     bias=0.0, scale=SCALE)
                if t >= 0:
                    nc.vector.tensor_mul(pth[:, q0:q0 + 128],
                                         pth[:, q0:q0 + 128], trimask[:])
                pts[(i, h)] = (pth, q0)

        def av_dn(i):
            for h in range(2):
                pth, q0 = pts.pop((i, h))
                mm(av[h][:, q0:JW], vs[i][:], pth[:, q0:JW],
                   start=(i == 0), stop=(i == nblk - 1))
                mm(dn_sl[h][:, q0:JW], onescol[:], pth[:, q0:JW],
                   start=(i == 0), stop=(i == nblk - 1))

        st_exp(0)
        for i in range(1, nblk):
            st_exp(i)
            av_dn(i - 1)
        # the final av_dn is deferred past the first performer feature
        # matmuls so the B drain never leaves the PE idle

        def softmax_norm(avcs):
            res = []
            for h in range(2):
                r = sm_p.tile([1, JW], F32, name="rcs", tag="rcs", bufs=2)
                nc.scalar.activation(r[:], dn_sl[h], AF.Ln, bias=0.0, scale=1.0)
                rb = sm_p.tile([1, JW], BF16, name="rcb", tag="rcb", bufs=2)
                nc.scalar.activation(rb[:], r[:], AF.Exp, bias=0.0, scale=-1.0)
                bb = smt([128, JW], F32, "bbs")
                mm(bb[:], onesr[:], rb[:], start=True, stop=True)
                a = at_p.tile([128, JW], BF16, name=f"ats{h}", tag=f"ats{h}",
                              bufs=2)
                nc.vector.tensor_mul(a[:], avcs[h][:], bb[:])
                res.append(a)
            return res

        # ================= C: performer heads (+ interleaved O-proj) ======
        atp01 = at_p.tile([128, 1024], BF16, name="atp01", tag="atp01", bufs=2)
        q2J = qt_p.tile([128, 1024], BF16, name="q2J", tag="q2J", bufs=2)
        nc.vector.tensor_mul(q2J[:], qt01[:], qt01[:])
        k2J = qt_p.tile([128, JW], BF16, name="k2J", tag="k2J", bufs=2)
        nc.vector.tensor_mul(k2J[:], ktp[:, s0:s0 + JW], ktp[:, s0:s0 + JW])
        feat = {}
        bias_d = {}

        def c_feat(t):
            c = 4 * J + t
            fq01 = ppt([128, 264])
            for h in range(2):
                qo = h * 512 + t * 128
                mm(fq01[:, h * 132:h * 132 + 128], qt01[:, qo:qo + 128],
                   omgx[:], start=True, stop=True)
                mm(fq01[:, h * 132 + 128:h * 132 + 130], q2J[:, qo:qo + 128],
                   cons2[:], start=True, stop=True)
            fk = ppt([128, 132])
            mm(fk[:, 0:128], ktp[:, c * 128:(c + 1) * 128], omgx[:],
               start=True, stop=True)
            mm(fk[:, 128:130], k2J[:, t * 128:(t + 1) * 128], cons2[:],
               start=True, stop=True)
            feat[t] = (fq01, fk)

        def c_bias(t):
            fq01, fk = feat.pop(t)
            f3 = fq01.rearrange("p (h c) -> p h c", h=2)
            nmax = sm_p.tile([128, 2], F32, name="nmax", tag="nmax", bufs=2)
            nc.vector.tensor_reduce(nmax[:], f3[:, :, 0:128], axis=AX.X,
                                    op=ALU.max, negate=True)
            nbq = sm_p.tile([128, 2], F32, name="nbq", tag="nbq", bufs=2)
            nc.vector.tensor_tensor(nbq[:], nmax[:],
                                    f3[:, :, 128:129].squeeze(-1),
                                    op=ALU.subtract)
            nc.vector.tensor_scalar(nbq[:], nbq[:], 1.0, -LNM,
                                    ALU.mult, ALU.add)
            nbk = sm_p.tile([128, 1], F32, name="nbk", tag="nbk", bufs=2)
            nc.vector.tensor_scalar(nbk[:], fk[:, 128:129], -1.0,
                                    nbinit[:, 0:1], ALU.mult, ALU.add)
            pq01 = sm_p.tile([128, 256], BF16, name="pq01", tag="pq01", bufs=2)
            for h in range(2):
                nc.scalar.activation(pq01[:, h * 128:(h + 1) * 128],
                                     fq01[:, h * 132:h * 132 + 128], AF.Exp,
                                     bias=nbq[:, h:h + 1], scale=1.0)
            pk = sm_p.tile([128, 128], BF16, name="pk", tag="pk", bufs=2)
            nc.scalar.activation(pk[:], fk[:, 0:128], AF.Exp, bias=nbk[:],
                                 scale=1.0)
            bias_d[t] = (pq01, pk)

        def c_attn(t, Jc, pqpk):
            # token-major numerators/denominators: the per-token divide is a
            # [128,2] column reciprocal + per-partition tensor_scalar, then
            # two PE transposes bring the result back to feature-major.
            nonlocal kv_bf
            c = 4 * Jc + t
            pq01, pk = pqpk
            trq01 = smt([128, 256], BF16, "trq")
            for h in range(2):
                nc.tensor.transpose(trq01[:, h * 128:(h + 1) * 128],
                                    pq01[:, h * 128:(h + 1) * 128], ident[:])
            trk = smt([128, 128], BF16, "trk")
            nc.tensor.transpose(trk[:], pk[:], ident[:])
            pqT01 = sm_p.tile([128, 256], BF16, name="pqT01", tag="pqT01", bufs=2)
            nc.vector.tensor_copy(pqT01[:], trq01[:])
            pkT = sm_p.tile([128, 128], BF16, name="pkT", tag="pkT", bufs=2)
            nc.vector.tensor_copy(pkT[:], trk[:])
            kvc = smt([128, 132], F32, "kvc")
            mm(kvc[:, 0:129], pk[:], vp[c][:, 0:129], start=True, stop=True)
            aT01 = smt([128, 256], F32, "aT")
            mm(aT01[:], pkT[:], pqT01[:], start=True, stop=True)
            aM01 = sm_p.tile([128, 256], BF16, name="aM01", tag="aM01", bufs=2)
            nc.vector.tensor_tensor(
                aM01.rearrange("p (h q) -> p h q", h=2),
                aT01.rearrange("p (h q) -> p h q", h=2),
                trimask.unsqueeze(1).broadcast_to([128, 2, 128]),
                op=ALU.mult)
            numt = smt([128, 256], F32, "numt")      # [q, h*hd] token-major
            dnpt = smt([128, 2], F32, "dnpt")        # [q, h] token-major
            for h in range(2):
                hs_ = slice(h * 128, (h + 1) * 128)
                mm(numt[:, hs_], aM01[:, hs_], vp[c][:, 0:128],
                   start=True, stop=False)
                mm(dnpt[:, h:h + 1], aM01[:, hs_], onescol[:],
                   start=True, stop=False)
                mm(numt[:, hs_], pqT01[:, hs_], kv_bf[:, 0:128],
                   start=False, stop=True)
                mm(dnpt[:, h:h + 1], pqT01[:, hs_], kv_bf[:, 128:129],
                   start=False, stop=True)
            nkv = sm_p.tile([128, 132], BF16, name="kvbf", tag="kvbf", bufs=2)
            nc.vector.tensor_add(nkv[:, 0:129], kv_bf[:, 0:129], kvc[:, 0:129])
            kv_bf = nkv
            numc = sm_p.tile([128, 256], BF16, name="numc", tag="numc", bufs=2)
            nc.scalar.copy(numc[:], numt[:])
            dent = sm_p.tile([128, 2], F32, name="dent", tag="dent", bufs=2)
            nc.vector.tensor_scalar(dent[:], dnpt[:], 1.0, nbinit[:, 2:3],
                                    ALU.mult, ALU.add)
            nc.vector.reciprocal(dent[:], dent[:])
            att = sm_p.tile([128, 256], BF16, name="att", tag="att", bufs=2)
            for h in range(2):
                hs_ = slice(h * 128, (h + 1) * 128)
                nc.vector.tensor_scalar_mul(att[:, hs_], numc[:, hs_],
                                            dent[:, h:h + 1])
            return att

        def c_fin(t, att, atp01c):
            # transpose token-major attention back to feature-major atp01
            cs = t * 128
            atr = smt([128, 256], BF16, "atr")
            for h in range(2):
                nc.tensor.transpose(atr[:, h * 128:(h + 1) * 128],
                                    att[:, h * 128:(h + 1) * 128], ident[:])
            nc.vector.tensor_copy(
                atp01c.rearrange("p (h s) -> p h s", h=2)[:, :, cs:cs + 128],
                atr.rearrange("p (h q) -> p h q", h=2))

        last = J == NJ - 1
        c_feat(0)
        av_dn(nblk - 1)
        avcs = []
        for h in range(2):
            avc = sm_p.tile([128, JW], BF16, name="avc", tag="avc", bufs=2)
            nc.vector.tensor_copy(avc[:], av[h][:])
            avcs.append(avc)
        ats = None
        # o-projection blocks of the previous window act as PE filler; two go
        # right at the B->C boundary where the ACT queue is most congested
        for t in range(4):
            if t == 0:
                if prev_at is not None:
                    oproj_sb(J - 1, prev_at, 0)
                else:
                    vproj_sb(1, hst_n, 0, filler=True)
            c_bias(t)
            if t + 1 < 4:
                c_feat(t + 1)
            if ats is None:
                # after bias(0) so its Ln/Exp follows the performer exps
                ats = softmax_norm(avcs)
            if t < 3:
                if prev_at is not None:
                    oproj_sb(J - 1, prev_at, t + 1)
                else:
                    vproj_sb(1, hst_n, t + 1, filler=True)
            if t < 3 or last:
                att = c_attn(t, J, bias_d.pop(t))
                c_fin(t, att, atp01)
                if last:
                    oproj_sb(J, (atp01, ats[0], ats[1]), t)

        if not last:
            def mk_pend(Jp, atp01p, pqpk, fattn, ffin):
                box = {}

                def run_attn():
                    box["att"] = fattn(3, Jp, pqpk)

                def run_fin():
                    ffin(3, box.pop("att"), atp01p)

                return (run_attn, run_fin)

            pend = mk_pend(J, atp01, bias_d.pop(3), c_attn, c_fin)
        prev_at = (atp01, ats[0], ats[1])


def _pin_act_tables():
    """Make every ACT table-set except natural_log_exp_and_others ineligible so
    the loader never thrashes between the exp-only and ln-only sets."""
    import concourse.bacc as bacc_mod
    if getattr(bacc_mod, "_act_tables_pinned", False):
        return
    orig = bacc_mod.get_activation_tables

    def patched(arch):
        t = orig(arch)
        return {k: (v if k == "natural_log_exp_and_others" else set())
                for k, v in t.items()}

    bacc_mod.get_activation_tables = patched
    bacc_mod._act_tables_pinned = True


def build(debug=False):
    _pin_act_tables()
    nc = bacc.Bacc("TRN2", target_bir_lowering=False, debug=False, num_devices=8)
    shapes = {
        "hsT": [D, S], "wq": [D, 512], "wk": [D, 256], "wv": [D, 256],
        "wo": [512, D], "cost": [128, S], "sintn": [128, S],
        "omgx": [128, 128], "cons2": [128, 2], "ident": [128, 128],
        "trimask": [128, 128], "onescol": [128, 1],
    }
    aps = {n: nc.dram_tensor(n, s, BF16, kind="ExternalInput").ap()
           for n, s in shapes.items()}
    aps["nbinit"] = nc.dram_tensor("nbinit", [128, 4], F32,
                                   kind="ExternalInput").ap()
    aps["onesr"] = nc.dram_tensor("onesr", [1, 128], BF16,
                                  kind="ExternalInput").ap()
    aps["out"] = nc.dram_tensor("out", [S, D], BF16, kind="ExternalOutput").ap()
    with tile.TileContext(nc) as tc:
        _emit(tc, aps, debug=debug)
    nc.compile()
    return nc


def host_prep(hidden_states, cos, sin, Wq, Wk, Wv, Wo, omega):
    """Slice/transpose/cast full inputs into 8 per-core input maps."""
    import ml_dtypes
    bf = ml_dtypes.bfloat16
    f32 = np.float32
    hs = np.asarray(hidden_states, f32)
    cos = np.asarray(cos, f32)
    sin = np.asarray(sin, f32)
    Wq, Wk, Wv, Wo = (np.asarray(x, f32) for x in (Wq, Wk, Wv, Wo))
    omega = np.asarray(omega, f32)

    omgx = np.ascontiguousarray((omega * HDQ).T).astype(bf)       # [hd, m]
    cons2 = np.zeros((128, 2), f32)
    cons2[:, 0] = 0.5 * HD ** -0.5
    cons2 = cons2.astype(bf)
    ident = np.eye(128, dtype=f32).astype(bf)
    pidx = np.arange(128)[:, None]
    qidx = np.arange(128)[None, :]
    trimask = (qidx >= pidx).astype(f32).astype(bf)                # keep q>=k
    onescol = np.ones((128, 1), f32).astype(bf)

    # stabk per (b, perf kv head j): max over (s,m) of projk (pre-stab)
    stab = np.zeros((B, 4), f32)
    kproj = np.einsum("bsd,od->bso", hs, Wk[0:512]).reshape(B, S, 4, HD)
    khalf = np.concatenate([-kproj[..., 64:], kproj[..., :64]], axis=-1)
    krot = kproj * cos[:, :, None, :] + khalf * sin[:, :, None, :]
    for b in range(B):
        for j in range(4):
            pj = (krot[b, :, j] * HDQ) @ omega.T
            stab[b, j] = pj.max()

    in_maps = []
    for core in range(8):
        b, j = divmod(core, 4)
        heads = [2 * j, 2 * j + 1, 8 + 2 * j, 8 + 2 * j + 1]
        qrows = np.concatenate([Wq[h * 128:(h + 1) * 128] for h in heads])
        kvh = [j, 4 + j]
        krows = np.concatenate([Wk[g * 128:(g + 1) * 128] for g in kvh])
        vrows = np.concatenate([Wv[g * 128:(g + 1) * 128] for g in kvh])
        wocols = np.concatenate([Wo[:, h * 128:(h + 1) * 128] for h in heads],
                                axis=1)
        sh = sin[b, :, 0:64]
        sintn = np.ascontiguousarray(np.concatenate([-sh, sh], axis=1).T)
        nbinit = np.zeros((128, 4), f32)
        nbinit[:, 0] = -(stab[b, j] + LNM)
        nbinit[:, 2] = EPS
        in_maps.append({
            "hsT": np.ascontiguousarray(hs[b].T).astype(bf),
            "wq": np.ascontiguousarray(qrows.T).astype(bf),
            "wk": np.ascontiguousarray(krows.T).astype(bf),
            "wv": np.ascontiguousarray(vrows.T).astype(bf),
            "wo": np.ascontiguousarray(wocols.T).astype(bf),
            "cost": np.ascontiguousarray(cos[b].T).astype(bf),
            "sintn": sintn.astype(bf),
            "omgx": omgx, "cons2": cons2, "ident": ident,
            "trimask": trimask, "onescol": onescol,
            "nbinit": nbinit,
            "onesr": np.ones((1, 128), f32).astype(bf),
        })
    return in_maps


_NC_CACHE = {}


def kernel(**inputs):
    from concourse.bass_utils import run_bass_kernel_spmd
    if "nc" not in _NC_CACHE:
        _NC_CACHE["nc"] = build(debug=False)
    nc = _NC_CACHE["nc"]
    in_maps = host_prep(**inputs)
    res = run_bass_kernel_spmd(nc, in_maps, core_ids=list(range(8)))
    out = np.zeros((B, S, D), np.float32)
    for core in range(8):
        out[core // 4] += res.results[core]["out"].astype(np.float32)
    return out


# revision 51
# speedup vs baseline: 1.0180x; 1.0180x over previous
"""Trainium2 Bass kernel for MixedPerformerAttention (B=2,S=2048,D=2048,H=16).

Sharding: 8 cores = 2 batches x 4 head-slots. Core c (b=c//4, j=c%4) owns
performer heads {2j, 2j+1} (kv head j) and softmax heads {8+2j, 8+2j+1}
(kv head 4+j), plus the matching Wq/Wk/Wv rows and Wo columns. Each core
computes a [S, D] partial output projection; the host sums 4 partials/batch.

All matmul operands are bf16 (fp32 PSUM accumulation). The two performer
heads share their GQA kv head, so k-features, pk and the entire kv prefix
state are computed once and reused by both heads; the q-side processes both
heads per chunk through merged [128,256] tiles (h-major) to amortize DVE
per-instruction overhead. Reciprocals run on DVE (vector.reciprocal), and
the performer per-token 1/den broadcast is a rank-1 f32r matmul instead of
gpsimd partition_broadcast.

Emission is software-pipelined: scores exp(i) overlaps AV(i-1) matmuls; the
performer chunks run FEAT/BIAS/ATTN stages with the O-projection of window
J-1 interleaved per chunk (sb=t) so its dense matmuls hide the performer's
DVE/ACT chains; for the last window its own O-projection chases each chunk.

The performer branch reproduces the reference's exact stabilizers (per-token
q-stab + per-(b,h) global k-stab) so the EPS=1e-6 denominator guard matches;
stabk is computed on the host at runtime and shipped in `nbinit`.

PSUM (8 banks): pp ring x2 (projections / scores / performer features),
av x2 (softmax accumulators), sm ring x2 (transposes/aT/kvc/num/bb/pso),
dacc0 (softmax den h0 row + performer den row), dacc1 (softmax den h1).
"""

import sys

sys.path.insert(0, "/opt/trn_rl_repo")

import numpy as np

import concourse.bass as bass
import concourse.tile as tile
from concourse import bacc, mybir
from concourse._compat import with_exitstack

F32 = mybir.dt.float32
F32R = mybir.dt.float32r
BF16 = mybir.dt.bfloat16
AF = mybir.ActivationFunctionType
AX = mybir.AxisListType
ALU = mybir.AluOpType

B, S, D = 2, 2048, 2048
H, KVH, HD = 16, 8, 128
NPH, M, C = 8, 128, 128
SCALE = HD ** -0.5
EPS = 1e-6
LNM = float(np.log(np.sqrt(M)))
HDQ = HD ** -0.25

NJ, JW, ND, NB = 4, 512, 16, 16


@with_exitstack
def _emit(ctx, tc, aps, debug=False):
    nc = tc.nc
    hsT, wq, wk, wv, wo = aps["hsT"], aps["wq"], aps["wk"], aps["wv"], aps["wo"]
    out = aps["out"]

    pers = ctx.enter_context(tc.tile_pool(name="pers", bufs=1))
    hst_p = ctx.enter_context(tc.tile_pool(name="hst", bufs=2))
    rot_p = ctx.enter_context(tc.tile_pool(name="rot", bufs=2))
    qt_p = ctx.enter_context(tc.tile_pool(name="qt", bufs=2))
    pt_p = ctx.enter_context(tc.tile_pool(name="pt", bufs=2))
    at_p = ctx.enter_context(tc.tile_pool(name="at", bufs=2))
    sm_p = ctx.enter_context(tc.tile_pool(name="sm", bufs=2))
    ost_p = ctx.enter_context(tc.tile_pool(name="ost", bufs=2))
    psp = ctx.enter_context(tc.tile_pool(name="psp", bufs=1, space="PSUM"))

    def ppt(shape=None, tag="pp"):
        return psp.tile(shape or [128, JW], F32, name=tag, tag="pp", bufs=2)

    def smt(shape, dt, name):
        return psp.tile(shape, dt, name=name, tag="sm", bufs=2)

    mm = nc.tensor.matmul

    # ---- weights + J0 activations, batched DMAs interleaved so the first
    # projection matmuls start after one chunk; tiny constants (needed only
    # by B/C) issue after the first compute-critical chunks ----
    wq_t = pers.tile([128, ND * 512], BF16, name="wq_t", tag="wq")
    wk_t = pers.tile([128, ND * 256], BF16, name="wk_t", tag="wk")
    wv_t = pers.tile([128, ND * 256], BF16, name="wv_t", tag="wv")
    wo_t = pers.tile([128, 4 * D], BF16, name="wo_t", tag="wo")
    hst0 = hst_p.tile([128, ND * JW], BF16, name="hst", tag="hst")
    wq_src = wq.rearrange("(d p) c -> p d c", p=128)
    hs_3 = hst0.rearrange("p (d s) -> p d s", d=ND)
    for dd in range(4):
        sl = slice(dd * 4, (dd + 1) * 4)
        nc.sync.dma_start(wq_t.rearrange("p (d c) -> p d c", d=ND)[:, sl, :],
                          wq_src[:, sl, :])
        nc.sync.dma_start(hs_3[:, sl, :],
                          hsT[:, 0:JW].rearrange("(d p) s -> p d s", p=128)[:, sl, :])
    co0 = rot_p.tile([128, JW], BF16, name="cos", tag="cos")
    si0 = rot_p.tile([128, JW], BF16, name="sin", tag="sin")
    nc.sync.dma_start(co0[:], aps["cost"][:, 0:JW])
    nc.sync.dma_start(si0[:], aps["sintn"][:, 0:JW])
    nc.sync.dma_start(wk_t.rearrange("p (d c) -> p d c", d=ND),
                      wk.rearrange("(d p) c -> p d c", p=128))
    nc.sync.dma_start(wv_t.rearrange("p (d c) -> p d c", d=ND),
                      wv.rearrange("(d p) c -> p d c", p=128))
    omgx = pers.tile([128, 128], BF16, name="omgx", tag="omgx")
    nc.sync.dma_start(omgx[:], aps["omgx"][:])
    cons2 = pers.tile([128, 2], BF16, name="cons2", tag="cons2")
    nc.sync.dma_start(cons2[:], aps["cons2"][:])
    ident = pers.tile([128, 128], BF16, name="ident", tag="ident")
    nc.sync.dma_start(ident[:], aps["ident"][:])
    trimask = pers.tile([128, 128], BF16, name="trimask", tag="trimask")
    nc.sync.dma_start(trimask[:], aps["trimask"][:])
    onescol = pers.tile([128, 1], BF16, name="onescol", tag="onescol")
    nc.sync.dma_start(onescol[:], aps["onescol"][:])
    onesr = pers.tile([1, 128], BF16, name="onesr", tag="onesr")
    nc.sync.dma_start(onesr[:], aps["onesr"][:])
    nbinit = pers.tile([128, 4], F32, name="nbinit", tag="nbinit")
    nc.sync.dma_start(nbinit[:], aps["nbinit"][:])
    nc.sync.dma_start(wo_t.rearrange("p (i c) -> p i c", i=4),
                      wo.rearrange("(i p) c -> p i c", p=128))

    # ---- persistent K/V and performer state ----
    ktp = pers.tile([128, 2048], BF16, name="ktp", tag="ktp")
    kts = pers.tile([128, 2048], BF16, name="kts", tag="kts")
    vp = [pers.tile([128, 132], BF16, name=f"vp{i}", tag=f"vp{i}") for i in range(NB)]
    vs = [pers.tile([128, 128], BF16, name=f"vs{i}", tag=f"vs{i}") for i in range(NB)]
    for i in range(NB):
        nc.vector.memset(vp[i][:, 128:129], 1.0)
    kv_bf = sm_p.tile([128, 132], BF16, name="kvbf", tag="kvbf", bufs=2)
    nc.vector.memset(kv_bf[:, 0:129], 0.0)

    # softmax denominator banks, one per head
    dh = [psp.tile([128, 512], F32, name=f"dacc{h}", tag=f"dacc{h}", bufs=1)
          for h in range(2)]
    dn_sl = [dh[h][0:1, :] for h in range(2)]

    def rotary(ps, dst, co, si):
        # dst = ps*cos + rot_half(ps)*sin; sintn has [-s; s] baked in. The
        # half-swapped products read PSUM directly (mixed PSUM/SB operands
        # are exempt from the same-base-partition rule).
        tmp = rot_p.tile([128, JW], BF16, name="rtmp", tag="rtmp", bufs=2)
        nc.vector.tensor_mul(tmp[0:64, :], ps[64:128, :], si[0:64, :])
        nc.vector.tensor_mul(tmp[64:128, :], ps[0:64, :], si[64:128, :])
        pc = rot_p.tile([128, JW], BF16, name="pc", tag="pc", bufs=2)
        nc.scalar.copy(pc[:], ps[:])
        nc.vector.tensor_mul(dst, pc[:], co[:])
        nc.vector.tensor_add(dst, dst, tmp[:])

    def oproj_sb(Jp, at3, sb):
        atp01p, ats0p, ats1p = at3
        s0p = Jp * JW
        o = ost_p.tile([128, D], BF16, name="ost", tag="ost", bufs=2)
        asl = [atp01p[:, sb * 128:sb * 128 + 128],
               atp01p[:, 512 + sb * 128:512 + sb * 128 + 128],
               ats0p[:, sb * 128:(sb + 1) * 128],
               ats1p[:, sb * 128:(sb + 1) * 128]]
        for oc in range(4):
            pso = smt([128, JW], F32, "pso")
            for i in range(4):
                mm(pso[:], asl[i], wo_t[:, i * D + oc * 512:i * D + (oc + 1) * 512],
                   start=(i == 0), stop=(i == 3))
            if oc % 2 == 0:
                nc.vector.tensor_copy(o[:, oc * 512:(oc + 1) * 512], pso[:])
            else:
                nc.scalar.copy(o[:, oc * 512:(oc + 1) * 512], pso[:])
                half = slice((oc - 1) * 512, (oc + 1) * 512)
                nc.sync.dma_start(
                    out[s0p + sb * 128:s0p + (sb + 1) * 128, half], o[:, half])

    prev_at = None

    pend = None  # deferred last performer unit of the previous window

    for J in range(NJ):
        s0 = J * JW
        if J == 0:
            hst, co, si = hst0, co0, si0
        else:
            hst, co, si = hst_n, co_n, si_n

        # ================= A: projections =================
        # The previous window's last performer unit is emitted between
        # projection groups: its cross-engine chains resolved long ago, so
        # it can never head-of-line-block this window's dense matmuls.
        qt01 = qt_p.tile([128, 1024], BF16, name="qt01", tag="qt01", bufs=2)
        qt2 = qt_p.tile([128, JW], BF16, name="qt2", tag="qt2", bufs=2)
        qt3 = qt_p.tile([128, JW], BF16, name="qt3", tag="qt3", bufs=2)
        qdst = [qt01[:, 0:512], qt01[:, 512:1024], qt2[:], qt3[:]]
        for g in range(4):
            ps = ppt()
            for d in range(ND):
                mm(ps[:], wq_t[:, d * 512 + g * 128:d * 512 + (g + 1) * 128],
                   hst[:, d * JW:(d + 1) * JW], start=(d == 0), stop=(d == ND - 1))
            rotary(ps, qdst[g], co, si)
            if g == 1 and pend is not None:
                pend[0]()  # attn of previous window's t=3
        for g in range(2):
            ps = ppt()
            for d in range(ND):
                mm(ps[:], wk_t[:, d * 256 + g * 128:d * 256 + (g + 1) * 128],
                   hst[:, d * JW:(d + 1) * JW], start=(d == 0), stop=(d == ND - 1))
            kt = ktp if g == 0 else kts
            rotary(ps, kt[:, s0:s0 + JW], co, si)
            if g == 1 and pend is not None:
                pend[1]()  # fin of previous window's t=3
                pend = None
        def vproj_sb(Jv, hstv, sb, filler=False):
            blk = 4 * Jv + sb
            # filler blocks run inside the performer section and must not
            # touch the pp ring (its tiles have not been consumed yet there)
            ps = smt([128, 256], F32, "psv") if filler else ppt([128, 256])
            for d in range(ND):
                mm(ps[:], hstv[:, d * JW + sb * 128:d * JW + (sb + 1) * 128],
                   wv_t[:, d * 256:(d + 1) * 256], start=(d == 0), stop=(d == ND - 1))
            nc.scalar.copy(vp[blk][:, 0:128], ps[:, 0:128])
            nc.scalar.copy(vs[blk][:], ps[:, 128:256])

        for sb in range(4):
            vproj_sb(J, hst, sb)

        # prefetch next window's activations while B/C run
        if J + 1 < NJ:
            s1 = (J + 1) * JW
            hst_n = hst_p.tile([128, ND * JW], BF16, name="hst", tag="hst")
            nc.sync.dma_start(
                hst_n.rearrange("p (d s) -> p d s", d=ND),
                hsT[:, s1:s1 + JW].rearrange("(d p) s -> p d s", p=128))
            co_n = rot_p.tile([128, JW], BF16, name="cos", tag="cos")
            si_n = rot_p.tile([128, JW], BF16, name="sin", tag="sin")
            nc.sync.dma_start(co_n[:], aps["cost"][:, s1:s1 + JW])
            nc.sync.dma_start(si_n[:], aps["sintn"][:, s1:s1 + JW])

        # ================= B: softmax heads =================
        nblk = 4 * J + 4
        av = [psp.tile([128, JW], F32, name=f"av{h}", tag="av", bufs=2)
              for h in range(2)]
        pts = {}

        def st_exp(i):
            t = i - 4 * J  # >= 0 on diagonal blocks
            q0 = max(t, 0) * 128
            for h in range(2):
                st = ppt()
                mm(st[:, q0:JW], kts[:, i * 128:(i + 1) * 128],
                   (qt2 if h == 0 else qt3)[:, q0:JW], start=True, stop=True)
                pth = pt_p.tile([128, JW], BF16, name=f"pt{h}", tag=f"pt{h}",
                                bufs=2)
                nc.scalar.activation(pth[:, q0:JW], st[:, q0:JW], AF.Exp,
                                     bias=0.0, scale=SCALE)
                if t >= 0:
                    nc.vector.tensor_mul(pth[:, q0:q0 + 128],
                                         pth[:, q0:q0 + 128], trimask[:])
                pts[(i, h)] = (pth, q0)

        def av_dn(i):
            for h in range(2):
                pth, q0 = pts.pop((i, h))
                mm(av[h][:, q0:JW], vs[i][:], pth[:, q0:JW],
                   start=(i == 0), stop=(i == nblk - 1))
                mm(dn_sl[h][:, q0:JW], onescol[:], pth[:, q0:JW],
                   start=(i == 0), stop=(i == nblk - 1))

        st_exp(0)
        for i in range(1, nblk):
            st_exp(i)
            av_dn(i - 1)
        # the final av_dn is deferred past the first performer feature
        # matmuls so the B drain never leaves the PE idle

        def softmax_norm(avcs):
            res = []
            for h in range(2):
                r = sm_p.tile([1, JW], F32, name="rcs", tag="rcs", bufs=2)
                nc.scalar.activation(r[:], dn_sl[h], AF.Ln, bias=0.0, scale=1.0)
                rb = sm_p.tile([1, JW], BF16, name="rcb", tag="rcb", bufs=2)
                nc.scalar.activation(rb[:], r[:], AF.Exp, bias=0.0, scale=-1.0)
                bb = smt([128, JW], F32, "bbs")
                mm(bb[:], onesr[:], rb[:], start=True, stop=True)
                a = at_p.tile([128, JW], BF16, name=f"ats{h}", tag=f"ats{h}",
                              bufs=2)
                nc.vector.tensor_mul(a[:], avcs[h][:], bb[:])
                res.append(a)
            return res

        # ================= C: performer heads (+ interleaved O-proj) ======
        atp01 = at_p.tile([128, 1024], BF16, name="atp01", tag="atp01", bufs=2)
        q2J = qt_p.tile([128, 1024], BF16, name="q2J", tag="q2J", bufs=2)
        nc.vector.tensor_mul(q2J[:], qt01[:], qt01[:])
        k2J = qt_p.tile([128, JW], BF16, name="k2J", tag="k2J", bufs=2)
        nc.vector.tensor_mul(k2J[:], ktp[:, s0:s0 + JW], ktp[:, s0:s0 + JW])
        feat = {}
        bias_d = {}

        def c_feat(t):
            c = 4 * J + t
            fq01 = ppt([128, 264])
            for h in range(2):
                qo = h * 512 + t * 128
                mm(fq01[:, h * 132:h * 132 + 128], qt01[:, qo:qo + 128],
                   omgx[:], start=True, stop=True)
                mm(fq01[:, h * 132 + 128:h * 132 + 130], q2J[:, qo:qo + 128],
                   cons2[:], start=True, stop=True)
            fk = ppt([128, 132])
            mm(fk[:, 0:128], ktp[:, c * 128:(c + 1) * 128], omgx[:],
               start=True, stop=True)
            mm(fk[:, 128:130], k2J[:, t * 128:(t + 1) * 128], cons2[:],
               start=True, stop=True)
            feat[t] = (fq01, fk)

        def c_bias(t):
            fq01, fk = feat.pop(t)
            f3 = fq01.rearrange("p (h c) -> p h c", h=2)
            nmax = sm_p.tile([128, 2], F32, name="nmax", tag="nmax", bufs=2)
            nc.vector.tensor_reduce(nmax[:], f3[:, :, 0:128], axis=AX.X,
                                    op=ALU.max, negate=True)
            nbq = sm_p.tile([128, 2], F32, name="nbq", tag="nbq", bufs=2)
            nc.vector.tensor_tensor(nbq[:], nmax[:],
                                    f3[:, :, 128:129].squeeze(-1),
                                    op=ALU.subtract)
            nc.vector.tensor_scalar(nbq[:], nbq[:], 1.0, -LNM,
                                    ALU.mult, ALU.add)
            nbk = sm_p.tile([128, 1], F32, name="nbk", tag="nbk", bufs=2)
            nc.vector.tensor_scalar(nbk[:], fk[:, 128:129], -1.0,
                                    nbinit[:, 0:1], ALU.mult, ALU.add)
            pq01 = sm_p.tile([128, 256], BF16, name="pq01", tag="pq01", bufs=2)
            for h in range(2):
                nc.scalar.activation(pq01[:, h * 128:(h + 1) * 128],
                                     fq01[:, h * 132:h * 132 + 128], AF.Exp,
                                     bias=nbq[:, h:h + 1], scale=1.0)
            pk = sm_p.tile([128, 128], BF16, name="pk", tag="pk", bufs=2)
            nc.scalar.activation(pk[:], fk[:, 0:128], AF.Exp, bias=nbk[:],
                                 scale=1.0)
            bias_d[t] = (pq01, pk)

        def c_attn(t, Jc, pqpk):
            # token-major numerators/denominators: the per-token divide is a
            # [128,2] column reciprocal + per-partition tensor_scalar, then
            # two PE transposes bring the result back to feature-major.
            nonlocal kv_bf
            c = 4 * Jc + t
            pq01, pk = pqpk
            trq01 = smt([128, 256], BF16, "trq")
            for h in range(2):
                nc.tensor.transpose(trq01[:, h * 128:(h + 1) * 128],
                                    pq01[:, h * 128:(h + 1) * 128], ident[:])
            trk = smt([128, 128], BF16, "trk")
            nc.tensor.transpose(trk[:], pk[:], ident[:])
            pqT01 = sm_p.tile([128, 256], BF16, name="pqT01", tag="pqT01", bufs=2)
            nc.vector.tensor_copy(pqT01[:], trq01[:])
            pkT = sm_p.tile([128, 128], BF16, name="pkT", tag="pkT", bufs=2)
            nc.vector.tensor_copy(pkT[:], trk[:])
            kvc = smt([128, 132], F32, "kvc")
            mm(kvc[:, 0:129], pk[:], vp[c][:, 0:129], start=True, stop=True)
            aT01 = smt([128, 256], F32, "aT")
            mm(aT01[:], pkT[:], pqT01[:], start=True, stop=True)
            aM01 = sm_p.tile([128, 256], BF16, name="aM01", tag="aM01", bufs=2)
            nc.vector.tensor_tensor(
                aM01.rearrange("p (h q) -> p h q", h=2),
                aT01.rearrange("p (h q) -> p h q", h=2),
                trimask.unsqueeze(1).broadcast_to([128, 2, 128]),
                op=ALU.mult)
            numt = smt([128, 256], F32, "numt")      # [q, h*hd] token-major
            dnpt = smt([128, 2], F32, "dnpt")        # [q, h] token-major
            for h in range(2):
                hs_ = slice(h * 128, (h + 1) * 128)
                mm(numt[:, hs_], aM01[:, hs_], vp[c][:, 0:128],
                   start=True, stop=False)
                mm(dnpt[:, h:h + 1], aM01[:, hs_], onescol[:],
                   start=True, stop=False)
                mm(numt[:, hs_], pqT01[:, hs_], kv_bf[:, 0:128],
                   start=False, stop=True)
                mm(dnpt[:, h:h + 1], pqT01[:, hs_], kv_bf[:, 128:129],
                   start=False, stop=True)
            nkv = sm_p.tile([128, 132], BF16, name="kvbf", tag="kvbf", bufs=2)
            nc.vector.tensor_add(nkv[:, 0:129], kv_bf[:, 0:129], kvc[:, 0:129])
            kv_bf = nkv
            numc = sm_p.tile([128, 256], BF16, name="numc", tag="numc", bufs=2)
            nc.scalar.copy(numc[:], numt[:])
            dent = sm_p.tile([128, 2], F32, name="dent", tag="dent", bufs=2)
            nc.vector.tensor_scalar(dent[:], dnpt[:], 1.0, nbinit[:, 2:3],
                                    ALU.mult, ALU.add)
            nc.vector.reciprocal(dent[:], dent[:])
            att = sm_p.tile([128, 256], BF16, name="att", tag="att", bufs=2)
            for h in range(2):
                hs_ = slice(h * 128, (h + 1) * 128)
                nc.vector.tensor_scalar_mul(att[:, hs_], numc[:, hs_],
                                            dent[:, h:h + 1])
            return att

        def c_fin(t, att, atp01c):
            # transpose token-major attention back to feature-major atp01
            cs = t * 128
            atr = smt([128, 256], BF16, "atr")
            for h in range(2):
                nc.tensor.transpose(atr[:, h * 128:(h + 1) * 128],
                                    att[:, h * 128:(h + 1) * 128], ident[:])
            nc.vector.tensor_copy(
                atp01c.rearrange("p (h s) -> p h s", h=2)[:, :, cs:cs + 128],
                atr.rearrange("p (h q) -> p h q", h=2))

        last = J == NJ - 1
        c_feat(0)
        av_dn(nblk - 1)
        avcs = []
        for h in range(2):
            avc = sm_p.tile([128, JW], BF16, name="avc", tag="avc", bufs=2)
            nc.vector.tensor_copy(avc[:], av[h][:])
            avcs.append(avc)
        ats = None
        # o-projection blocks of the previous window act as PE filler; two go
        # right at the B->C boundary where the ACT queue is most congested
        for t in range(4):
            if t == 0 and prev_at is not None:
                oproj_sb(J - 1, prev_at, 0)
            c_bias(t)
            if t + 1 < 4:
                c_feat(t + 1)
            if ats is None:
                # after bias(0) so its Ln/Exp follows the performer exps
                ats = softmax_norm(avcs)
            if prev_at is not None and t < 3:
                oproj_sb(J - 1, prev_at, t + 1)
            if t < 3 or last:
                att = c_attn(t, J, bias_d.pop(t))
                c_fin(t, att, atp01)
                if last:
                    oproj_sb(J, (atp01, ats[0], ats[1]), t)

        if not last:
            def mk_pend(Jp, atp01p, pqpk, fattn, ffin):
                box = {}

                def run_attn():
                    box["att"] = fattn(3, Jp, pqpk)

                def run_fin():
                    ffin(3, box.pop("att"), atp01p)

                return (run_attn, run_fin)

            pend = mk_pend(J, atp01, bias_d.pop(3), c_attn, c_fin)
        prev_at = (atp01, ats[0], ats[1])


def _pin_act_tables():
    """Make every ACT table-set except natural_log_exp_and_others ineligible so
    the loader never thrashes between the exp-only and ln-only sets."""
    import concourse.bacc as bacc_mod
    if getattr(bacc_mod, "_act_tables_pinned", False):
        return
    orig = bacc_mod.get_activation_tables

    def patched(arch):
        t = orig(arch)
        return {k: (v if k == "natural_log_exp_and_others" else set())
                for k, v in t.items()}

    bacc_mod.get_activation_tables = patched
    bacc_mod._act_tables_pinned = True


def build(debug=False):
    _pin_act_tables()
    nc = bacc.Bacc("TRN2", target_bir_lowering=False, debug=False, num_devices=8)
    shapes = {
        "hsT": [D, S], "wq": [D, 512], "wk": [D, 256], "wv": [D, 256],
        "wo": [512, D], "cost": [128, S], "sintn": [128, S],
        "omgx": [128, 128], "cons2": [128, 2], "ident": [128, 128],
        "trimask": [128, 128], "onescol": [128, 1],
    }
    aps = {n: nc.dram_tensor(n, s, BF16, kind="ExternalInput").ap()
           for n, s in shapes.items()}
    aps["nbinit"] = nc.dram_tensor("nbinit", [128, 4], F32,
                                   kind="ExternalInput").ap()
    aps["onesr"] = nc.dram_tensor("onesr", [1, 128], BF16,
                                  kind="ExternalInput").ap()
    aps["out"] = nc.dram_tensor("out", [S, D], BF16, kind="ExternalOutput").ap()
    with tile.TileContext(nc) as tc:
        _emit(tc, aps, debug=debug)
    nc.compile()
    return nc


def host_prep(hidden_states, cos, sin, Wq, Wk, Wv, Wo, omega):
    """Slice/transpose/cast full inputs into 8 per-core input maps."""
    import ml_dtypes
    bf = ml_dtypes.bfloat16
    f32 = np.float32
    hs = np.asarray(hidden_states, f32)
    cos = np.asarray(cos, f32)
    sin = np.asarray(sin, f32)
    Wq, Wk, Wv, Wo = (np.asarray(x, f32) for x in (Wq, Wk, Wv, Wo))
    omega = np.asarray(omega, f32)

    omgx = np.ascontiguousarray((omega * HDQ).T).astype(bf)       # [hd, m]
    cons2 = np.zeros((128, 2), f32)
    cons2[:, 0] = 0.5 * HD ** -0.5
    cons2 = cons2.astype(bf)
    ident = np.eye(128, dtype=f32).astype(bf)
    pidx = np.arange(128)[:, None]
    qidx = np.arange(128)[None, :]
    trimask = (qidx >= pidx).astype(f32).astype(bf)                # keep q>=k
    onescol = np.ones((128, 1), f32).astype(bf)

    # stabk per (b, perf kv head j): max over (s,m) of projk (pre-stab)
    stab = np.zeros((B, 4), f32)
    kproj = np.einsum("bsd,od->bso", hs, Wk[0:512]).reshape(B, S, 4, HD)
    khalf = np.concatenate([-kproj[..., 64:], kproj[..., :64]], axis=-1)
    krot = kproj * cos[:, :, None, :] + khalf * sin[:, :, None, :]
    for b in range(B):
        for j in range(4):
            pj = (krot[b, :, j] * HDQ) @ omega.T
            stab[b, j] = pj.max()

    in_maps = []
    for core in range(8):
        b, j = divmod(core, 4)
        heads = [2 * j, 2 * j + 1, 8 + 2 * j, 8 + 2 * j + 1]
        qrows = np.concatenate([Wq[h * 128:(h + 1) * 128] for h in heads])
        kvh = [j, 4 + j]
        krows = np.concatenate([Wk[g * 128:(g + 1) * 128] for g in kvh])
        vrows = np.concatenate([Wv[g * 128:(g + 1) * 128] for g in kvh])
        wocols = np.concatenate([Wo[:, h * 128:(h + 1) * 128] for h in heads],
                                axis=1)
        sh = sin[b, :, 0:64]
        sintn = np.ascontiguousarray(np.concatenate([-sh, sh], axis=1).T)
        nbinit = np.zeros((128, 4), f32)
        nbinit[:, 0] = -(stab[b, j] + LNM)
        nbinit[:, 2] = EPS
        in_maps.append({
            "hsT": np.ascontiguousarray(hs[b].T).astype(bf),
            "wq": np.ascontiguousarray(qrows.T).astype(bf),
            "wk": np.ascontiguousarray(krows.T).astype(bf),
            "wv": np.ascontiguousarray(vrows.T).astype(bf),
            "wo": np.ascontiguousarray(wocols.T).astype(bf),
            "cost": np.ascontiguousarray(cos[b].T).astype(bf),
            "sintn": sintn.astype(bf),
            "omgx": omgx, "cons2": cons2, "ident": ident,
            "trimask": trimask, "onescol": onescol,
            "nbinit": nbinit,
            "onesr": np.ones((1, 128), f32).astype(bf),
        })
    return in_maps


_NC_CACHE = {}


def kernel(**inputs):
    from concourse.bass_utils import run_bass_kernel_spmd
    if "nc" not in _NC_CACHE:
        _NC_CACHE["nc"] = build(debug=False)
    nc = _NC_CACHE["nc"]
    in_maps = host_prep(**inputs)
    res = run_bass_kernel_spmd(nc, in_maps, core_ids=list(range(8)))
    out = np.zeros((B, S, D), np.float32)
    for core in range(8):
        out[core // 4] += res.results[core]["out"].astype(np.float32)
    return out
